# revision 3
# baseline (speedup 1.0000x reference)
"""Trainium2 Bass kernel for nn_Compute_all_u (embedding gather + batched affine dot).

For each voxel v:
    u[v, :] = C[e_v, 0, :] + x_v*C[e_v, 1, :] + y_v*C[e_v, 2, :] + z_v*C[e_v, 3, :]
where e_v = voxels_elements[v], (x,y,z) = all_voxels_centroids[v].

Strategy (v2, "broadcast-R"): shard the ELEMENT TABLE across the 8 cores
(62,500 elements each) and route voxels to the core owning their element.
Each element is then referenced ~16x per core (Poisson(16)), so the device
never needs data-dependent addressing: the host sorts voxels by element and
packs each element's voxels into ceil(L/8) groups of R=8 consecutive slots;
the device streams one (host-repeated) table row per group and broadcasts it
across the group's 8 slots with stride-0 DVE access patterns.

This removes the SWDGE dma_gather entirely - the v1 kernel was bottlenecked
at ~8.7ns/row of Q7 descriptor generation (1M rows / 4 queues = 2.26ms),
with DMA engines only ~14% busy. v2 is pure sequential DMA + DVE math.

Precision: all device math in fp16 (centroids/table cast on host). Measured
rel err ~1e-3 vs the f32 reference (gate is 2e-2): values are O(1) normals,
u ~ N(0, 4), fp16 eps 9.8e-4.

Device layout per core (tile t, partition p, group-in-partition c, slot r):
  group g = (t*128 + p)*CG + c,  slot s = g*R + r
  trow[t, p, c*12:(c+1)*12]       = packed coeff row of group g (12 fp16)
  cent[t, p, (c*R+r)*3 : +3]      = centroid of slot s (fp16)
  out [t, p, (c*R+r)*3 : +3]      = u of slot s (fp16)

Host prep per call: one 8M argsort by element, per-core bincount/cumsum to
assign slots, np.repeat to build the group row stream (~2.4x the 3MB table
slice), scatter centroids into slot order, un-permute outputs. Any voxel
whose slot would exceed the padded group capacity NG (>>80 sigma away for
the generated inputs) falls back to exact host math.
"""

import numpy as np

from concourse import bacc, bass, tile, mybir
from concourse.bass_utils import run_bass_kernel_spmd

N_VOXELS = 8_000_000
N_ELEM = 500_000
N_CORES = 8
EPC = N_ELEM // N_CORES     # 62,500 elements per core
R = 8                       # slots per group (one broadcast row each)
CG = 128                    # groups per partition per tile
NT = 10                     # tiles per core
NG = NT * 128 * CG          # 163,840 group capacity (E~152.2k, sigma~145)
NSLOT = NG * R              # 1,310,720 slots per core

f16 = mybir.dt.float16

# group-columns per tile handled by DVE; the rest go to the Pool (GpSimd)
# engine so both elementwise engines run concurrently.
CGV = 96


def build_nc(bufs: int = 4) -> bass.Bass:
    nc = bacc.Bacc("TRN2")
    trow_in = nc.declare_dram_parameter("trow", [NT, 128, CG * 12], f16, isOutput=False)
    cent_in = nc.declare_dram_parameter("cent", [NT, 128, CG * R * 3], f16, isOutput=False)
    out = nc.declare_dram_parameter("out", [NT, 128, CG * R * 3], f16, isOutput=True)

    mul = mybir.AluOpType.mult
    add = mybir.AluOpType.add

    with tile.TileContext(nc) as tc:
        with (
            tc.tile_pool(name="io", bufs=bufs) as io_pool,
            tc.tile_pool(name="tmp", bufs=2) as tmp_pool,
        ):
            for t in range(NT):
                trow_t = io_pool.tile([128, CG * 12], f16, tag="trow")
                nc.sync.dma_start(out=trow_t[:], in_=trow_in[t])
                cent_t = io_pool.tile([128, CG * R * 3], f16, tag="cent")
                nc.sync.dma_start(out=cent_t[:], in_=cent_in[t])

                tr = trow_t[:].rearrange("p (c d) -> p c d", d=12)
                cr = cent_t[:].rearrange("p (c r j) -> p c r j", r=R, j=3)

                for eng, c0, c1 in ((nc.vector, 0, CGV), (nc.gpsimd, CGV, CG)):
                    w = c1 - c0
                    u = io_pool.tile([128, w * R * 3], f16, tag=f"u{c0}")
                    tmp = tmp_pool.tile([128, w * R * 3], f16, tag=f"t{c0}")
                    ur = u[:].rearrange("p (c r j) -> p c r j", r=R, j=3)
                    tmr = tmp[:].rearrange("p (c r j) -> p c r j", r=R, j=3)

                    def row(k):  # coeff row k, broadcast over the R slot axis
                        return tr[:, c0:c1, 3 * k:3 * k + 3].unsqueeze(2).to_broadcast(
                            [128, w, R, 3]
                        )

                    def xyz(j):  # centroid component j, broadcast over k
                        return cr[:, c0:c1, :, j:j + 1].to_broadcast([128, w, R, 3])

                    eng.tensor_tensor(out=tmr, in0=xyz(0), in1=row(1), op=mul)
                    eng.tensor_tensor(out=ur, in0=row(0), in1=tmr, op=add)
                    eng.tensor_tensor(out=tmr, in0=xyz(1), in1=row(2), op=mul)
                    eng.tensor_tensor(out=ur, in0=ur, in1=tmr, op=add)
                    eng.tensor_tensor(out=tmr, in0=xyz(2), in1=row(3), op=mul)
                    eng.tensor_tensor(out=ur, in0=ur, in1=tmr, op=add)

                    nc.sync.dma_start(
                        out=out[t][:, c0 * R * 3:c1 * R * 3], in_=u[:]
                    )
    nc.finalize()
    return nc


_NC_CACHE: dict = {}


def _get_nc():
    key = (R, CG, NT)
    if key not in _NC_CACHE:
        _NC_CACHE[key] = build_nc()
    return _NC_CACHE[key]


def _prep_core(el, vox, coeffs16_c, cent16_full):
    """Build one core's device arrays from its (sorted) local element ids."""
    n = el.shape[0]
    counts = np.bincount(el, minlength=EPC)
    ngrp = (counts + (R - 1)) // R
    gbase = np.zeros(EPC, dtype=np.int64)
    np.cumsum(ngrp[:-1], out=gbase[1:])
    run_start = np.zeros(EPC, dtype=np.int64)
    np.cumsum(counts[:-1], out=run_start[1:])
    rank = np.arange(n, dtype=np.int64) - run_start[el]
    slot = gbase[el] * R + rank
    ok = slot < NSLOT

    trow_dev = np.zeros((NG, 12), dtype=np.float16)
    total_g = int(ngrp.sum())
    if total_g <= NG:
        trow_dev[:total_g] = np.repeat(coeffs16_c, ngrp, axis=0)
    else:
        trow_dev[:] = np.repeat(coeffs16_c, ngrp, axis=0)[:NG]

    cent_dev = np.zeros((NSLOT, 3), dtype=np.float16)
    cent_dev[slot[ok]] = cent16_full[vox[ok]]

    return (
        {
            "trow": trow_dev.reshape(NT, 128, CG * 12),
            "cent": cent_dev.reshape(NT, 128, CG * R * 3),
        },
        slot,
        ok,
    )


def kernel(all_coeffs, all_voxels_centroids, voxels_elements, _trace=False, **run_kwargs):
    nc = _get_nc()
    coeffs12 = np.asarray(all_coeffs, dtype=np.float32).reshape(N_ELEM, 12)
    coeffs16 = coeffs12.astype(np.float16)
    cent_full = np.asarray(all_voxels_centroids, dtype=np.float32)
    cent16 = cent_full.astype(np.float16)
    e_full = np.asarray(voxels_elements).astype(np.int64)

    order = np.argsort(e_full, kind="stable")
    es = e_full[order]
    bounds = np.searchsorted(es, np.arange(N_CORES + 1, dtype=np.int64) * EPC)

    in_maps, metas = [], []
    for c in range(N_CORES):
        lo, hi = int(bounds[c]), int(bounds[c + 1])
        vox = order[lo:hi]
        el = (es[lo:hi] - c * EPC).astype(np.int64)
        m, slot, ok = _prep_core(el, vox, coeffs16[c * EPC:(c + 1) * EPC], cent16)
        in_maps.append(m)
        metas.append((vox, slot, ok))

    res = run_bass_kernel_spmd(
        nc, in_maps, core_ids=list(range(N_CORES)), trace=_trace, **run_kwargs
    )

    full = np.empty((N_VOXELS, 3), dtype=np.float32)
    for c in range(N_CORES):
        vox, slot, ok = metas[c]
        u_slots = res.results[c]["out"].reshape(NSLOT, 3)
        full[vox[ok]] = u_slots[slot[ok]].astype(np.float32)
        bad = ~ok
        if bad.any():
            vb = vox[bad]
            cf = coeffs12[e_full[vb]].reshape(-1, 4, 3)
            xyz = cent_full[vb]
            full[vb] = cf[:, 0] + np.einsum("nd,ndk->nk", xyz, cf[:, 1:4])
    if _trace:
        return full, res
    return full


# revision 4
# speedup vs baseline: 2.0999x; 2.0999x over previous
"""Trainium2 Bass kernel for nn_Compute_all_u (embedding gather + batched affine dot).

For each voxel v:
    u[v, :] = C[e_v, 0, :] + x_v*C[e_v, 1, :] + y_v*C[e_v, 2, :] + z_v*C[e_v, 3, :]
where e_v = voxels_elements[v], (x,y,z) = all_voxels_centroids[v].

Strategy ("broadcast-R"): shard the ELEMENT TABLE across the 8 cores
(62,500 elements each) and route voxels to the core owning their element.
Each element is then referenced ~16x per core (Poisson(16)), so the device
never needs data-dependent addressing: the host sorts voxels by element and
packs each element's voxels into ceil(L/8) groups of R=8 consecutive slots;
the device streams one (host-repeated) table row per group and broadcasts it
across the group's 8 slots with stride-0 DVE access patterns.

This removes the SWDGE dma_gather entirely - the v1 kernel was bottlenecked
at ~8.7ns/row of Q7 descriptor generation (1M rows / 4 queues = 2.26ms),
with DMA engines only ~14% busy. Here everything is sequential DMA + DVE.

Layouts are PLANAR so every DVE operand has innermost stride 1 (the 2x_1P
fp16 perf mode requires step_x=+-1 / 4B alignment on all srcs and dst;
broadcasts live on outer axes where stride 0 is allowed):
  trow[t, p, dk, c]   dk = d*3+k       (12 planes of CG rows)
  cent[t, p, j, r, c] j in {x,y,z}     (3 planes of R x CG)
  out [t, p, k, r, c] k output compnt  (3 planes of R x CG)
with group g = (t*128 + p)*CG + c holding slots s = g*R + r.

Per tile the 6 fp16 DVE ops (out shape [128, 3, R, CG]) are:
  tmp = X(bcast k) * C1(bcast r);  u  = C0(bcast r) + tmp
  tmp = Y(bcast k) * C2(bcast r);  u += tmp
  tmp = Z(bcast k) * C3(bcast r);  u += tmp

Precision: fp16 throughout; measured rel err ~1e-3 vs the f32 reference
(gate 2e-2): values are O(1) normals, u ~ N(0, 4), fp16 eps 9.8e-4.

Host prep per call: one 8M argsort by element, per-core bincount/cumsum to
assign slots, np.repeat to build the group row stream (~2.4x the 3MB table
slice), scatter centroids into slot-planar order, un-permute outputs. Any
voxel whose slot would exceed the padded group capacity NG (27 sigma above
the mean for the generated inputs) falls back to exact host math.
"""

import numpy as np

from concourse import bacc, bass, tile, mybir
from concourse.bass_utils import run_bass_kernel_spmd

N_VOXELS = 8_000_000
N_ELEM = 500_000
N_CORES = 8
EPC = N_ELEM // N_CORES     # 62,500 elements per core
R = 8                       # slots per group (one broadcast row each)
CG = 122                    # groups per partition per tile (244B rows, 4B aligned)
NT = 10                     # tiles per core
NG = NT * 128 * CG          # 156,160 group capacity (E~152.2k, sigma~145)
NSLOT = NG * R              # 1,249,280 slots per core

f16 = mybir.dt.float16


def build_nc(bufs: int = 4) -> bass.Bass:
    nc = bacc.Bacc("TRN2")
    trow_in = nc.declare_dram_parameter("trow", [NT, 128, 12 * CG], f16, isOutput=False)
    cent_in = nc.declare_dram_parameter("cent", [NT, 128, 3 * R * CG], f16, isOutput=False)
    out = nc.declare_dram_parameter("out", [NT, 128, 3 * R * CG], f16, isOutput=True)

    mul = mybir.AluOpType.mult
    add = mybir.AluOpType.add

    with tile.TileContext(nc) as tc:
        with (
            tc.tile_pool(name="io", bufs=bufs) as io_pool,
            tc.tile_pool(name="tmp", bufs=2) as tmp_pool,
        ):
            for t in range(NT):
                trow_t = io_pool.tile([128, 12 * CG], f16, tag="trow")
                nc.sync.dma_start(out=trow_t[:], in_=trow_in[t])
                cent_t = io_pool.tile([128, 3 * R * CG], f16, tag="cent")
                nc.sync.dma_start(out=cent_t[:], in_=cent_in[t])

                u = io_pool.tile([128, 3 * R * CG], f16, tag="u")
                tmp = tmp_pool.tile([128, 3 * R * CG], f16, tag="t")

                tr = trow_t[:].rearrange("p (dk c) -> p dk c", c=CG)
                cr = cent_t[:].rearrange("p (j r c) -> p j r c", r=R, c=CG)
                ur = u[:].rearrange("p (k r c) -> p k r c", r=R, c=CG)
                tmr = tmp[:].rearrange("p (k r c) -> p k r c", r=R, c=CG)

                def rows(d):  # trow planes d*3..d*3+3, broadcast over r (outer)
                    return tr[:, 3 * d:3 * d + 3, :].unsqueeze(2).to_broadcast(
                        [128, 3, R, CG]
                    )

                def xyz(j):  # cent plane j, broadcast over k (outer)
                    return cr[:, j:j + 1, :, :].to_broadcast([128, 3, R, CG])

                nc.vector.tensor_tensor(out=tmr, in0=xyz(0), in1=rows(1), op=mul)
                nc.vector.tensor_tensor(out=ur, in0=rows(0), in1=tmr, op=add)
                nc.vector.tensor_tensor(out=tmr, in0=xyz(1), in1=rows(2), op=mul)
                nc.vector.tensor_tensor(out=ur, in0=ur, in1=tmr, op=add)
                nc.vector.tensor_tensor(out=tmr, in0=xyz(2), in1=rows(3), op=mul)
                nc.vector.tensor_tensor(out=ur, in0=ur, in1=tmr, op=add)

                nc.sync.dma_start(out=out[t], in_=u[:])
    nc.finalize()
    return nc


_NC_CACHE: dict = {}


def _get_nc():
    key = (R, CG, NT)
    if key not in _NC_CACHE:
        _NC_CACHE[key] = build_nc()
    return _NC_CACHE[key]


def _prep_core(el, vox, coeffs16_c, cent16_full):
    """Build one core's device arrays from its (sorted) local element ids."""
    n = el.shape[0]
    counts = np.bincount(el, minlength=EPC)
    ngrp = (counts + (R - 1)) // R
    gbase = np.zeros(EPC, dtype=np.int64)
    np.cumsum(ngrp[:-1], out=gbase[1:])
    run_start = np.zeros(EPC, dtype=np.int64)
    np.cumsum(counts[:-1], out=run_start[1:])
    rank = np.arange(n, dtype=np.int64) - run_start[el]
    slot = gbase[el] * R + rank
    ok = slot < NSLOT

    trow_flat = np.zeros((NG, 12), dtype=np.float16)
    total_g = int(ngrp.sum())
    if total_g <= NG:
        trow_flat[:total_g] = np.repeat(coeffs16_c, ngrp, axis=0)
    else:
        trow_flat[:] = np.repeat(coeffs16_c, ngrp, axis=0)[:NG]

    cent_slot = np.zeros((NSLOT, 3), dtype=np.float16)
    cent_slot[slot[ok]] = cent16_full[vox[ok]]

    # planar device layouts (innermost = group axis c)
    trow_dev = np.ascontiguousarray(
        trow_flat.reshape(NT, 128, CG, 12).transpose(0, 1, 3, 2)
    ).reshape(NT, 128, 12 * CG)
    cent_dev = np.ascontiguousarray(
        cent_slot.reshape(NT, 128, CG, R, 3).transpose(0, 1, 4, 3, 2)
    ).reshape(NT, 128, 3 * R * CG)

    return (
        {"trow": trow_dev, "cent": cent_dev},
        slot,
        ok,
    )


def kernel(all_coeffs, all_voxels_centroids, voxels_elements, _trace=False, **run_kwargs):
    nc = _get_nc()
    coeffs12 = np.asarray(all_coeffs, dtype=np.float32).reshape(N_ELEM, 12)
    coeffs16 = coeffs12.astype(np.float16)
    cent_full = np.asarray(all_voxels_centroids, dtype=np.float32)
    cent16 = cent_full.astype(np.float16)
    e_full = np.asarray(voxels_elements).astype(np.int64)

    order = np.argsort(e_full, kind="stable")
    es = e_full[order]
    bounds = np.searchsorted(es, np.arange(N_CORES + 1, dtype=np.int64) * EPC)

    in_maps, metas = [], []
    for c in range(N_CORES):
        lo, hi = int(bounds[c]), int(bounds[c + 1])
        vox = order[lo:hi]
        el = (es[lo:hi] - c * EPC).astype(np.int64)
        m, slot, ok = _prep_core(el, vox, coeffs16[c * EPC:(c + 1) * EPC], cent16)
        in_maps.append(m)
        metas.append((vox, slot, ok))

    res = run_bass_kernel_spmd(
        nc, in_maps, core_ids=list(range(N_CORES)), trace=_trace, **run_kwargs
    )

    full = np.empty((N_VOXELS, 3), dtype=np.float32)
    for c in range(N_CORES):
        vox, slot, ok = metas[c]
        u_slots = np.ascontiguousarray(
            res.results[c]["out"].reshape(NT, 128, 3, R, CG).transpose(0, 1, 4, 3, 2)
        ).reshape(NSLOT, 3)
        full[vox[ok]] = u_slots[slot[ok]].astype(np.float32)
        bad = ~ok
        if bad.any():
            vb = vox[bad]
            cf = coeffs12[e_full[vb]].reshape(-1, 4, 3)
            xyz = cent_full[vb]
            full[vb] = cf[:, 0] + np.einsum("nd,ndk->nk", xyz, cf[:, 1:4])
    if _trace:
        return full, res
    return full
